# revision 12
# baseline (speedup 1.0000x reference)
"""Trainium2 Bass kernel for nn_Conv_39273180955616.

Computes, for X:(16,64,512,512) f32, K:(1,1,7,7), b:(1,1,1,1):
    out[n,c] = correlate2d(X[n,c], Keff, pad=3) + 49*b
where Keff = K.sum(axis=(0,1)).

Data parallel over the 1024 (n,c) planes -> 128 planes/core on 8 cores.

Per core the image planes are stored h-major in HBM (Xt[h, plane, w],
zero-padded to 518 in h and w) so one DMA descriptor carries a 16-plane
row run (16.6 KB).  The 7x7 correlation runs on TensorE as banded-
Toeplitz matmuls packed 4-per-pass onto the 128x128 PE array via 64x64
tile_position tiles: row half r in {0,64} holds a 64-row h-window (58
output rows), col half c in {0,64} computes a different plane, and the
7 kernel-column shifts accumulate in PSUM as free-dim offsets into the
518-wide padded rows.  Windows chain in pairs across the per-core
(block, window) list; the last window of each plane overlaps rows with
the previous one (band cols m<10 zeroed) so every window is full-size.
PSUM is evicted to SBUF as bf16 by DVE/ACT copies; stores go out over
the gpsimd SWDGE ring with 2 KB descriptors ([h, plane, w] bf16 layout,
plane pairs packed per partition).  The +49*b bias and the f32 cast are
applied on the host during the gather.
"""
import numpy as np
import ml_dtypes

import concourse.bass as bass
import concourse.tile as tile
from concourse import bacc, mybir
from concourse.bass_utils import run_bass_kernel_spmd

N_CORES = 8
H = 512
W = 512
HPAD = H + 6
WPAD = W + 6
PLANES = 16 * 64 // N_CORES     # 128 planes per core
BLK = 16                        # planes per block (descriptor run length)
NBLK = PLANES // BLK            # 8
WIN = 64                        # input rows per window
WOUT = 58                       # output rows per window (WIN - 6)
NWIN = 9                        # windows per plane: 8 @ stride 58 + 1 tail
TAIL_BASE = HPAD - WIN          # padded row base of tail window = 454
TAIL_MLO = 10                   # tail window valid outputs: m in [10, 58)


def _win_base(t):
    return 58 * t if t < 8 else TAIL_BASE


def _build_weights(Keff: np.ndarray) -> np.ndarray:
    """wb [128, 2*7*64] bf16: variant v (0 normal, 1 tail), dw in 0..6.

    B[p, m] = Keff[p-m, dw] for 0 <= p-m <= 6, m in [mlo, 58), else 0.
    Both 64-partition halves hold the same content.
    """
    wb = np.zeros((128, 2 * 7 * 64), np.float32)
    p = np.arange(64)[:, None]
    m = np.arange(64)[None, :]
    dh = p - m
    for v in range(2):
        mlo = TAIL_MLO if v == 1 else 0
        ok = (dh >= 0) & (dh < 7) & (m >= mlo) & (m < WOUT)
        for dw in range(7):
            blk = np.zeros((64, 64), np.float32)
            blk[ok] = Keff[dh[ok], dw]
            c0 = (v * 7 + dw) * 64
            wb[0:64, c0:c0 + 64] = blk
            wb[64:128, c0:c0 + 64] = blk
    return wb.astype(ml_dtypes.bfloat16)


_NC_CACHE = {}


def _get_module():
    if "nc" in _NC_CACHE:
        return _NC_CACHE["nc"]
    nc = bacc.Bacc("TRN2", target_bir_lowering=False, debug=False,
                   num_devices=N_CORES)
    xt = nc.dram_tensor("xt", [HPAD, PLANES, WPAD], mybir.dt.bfloat16,
                        kind="ExternalInput")
    wt = nc.dram_tensor("wt", [128, 2 * 7 * 64], mybir.dt.bfloat16,
                        kind="ExternalInput")
    out = nc.dram_tensor("out", [H, PLANES, W], mybir.dt.bfloat16,
                         kind="ExternalOutput")

    # flat (block, window) list; consecutive entries pair onto the two
    # 64-partition halves of one macrotile
    wis = [(b, t) for b in range(NBLK) for t in range(NWIN)]
    assert len(wis) % 2 == 0

    with tile.TileContext(nc) as tc:
        with (
            tc.tile_pool(name="wp", bufs=1) as wpool,
            tc.tile_pool(name="mt", bufs=3) as mtpool,
            tc.tile_pool(name="ps", bufs=3, space="PSUM") as pspool,
            tc.tile_pool(name="ev", bufs=4) as evpool,
        ):
            wb = wpool.tile([128, 2 * 7 * 64], mybir.dt.bfloat16)
            nc.sync.dma_start(wb[:], wt.ap())

            for pair in range(len(wis) // 2):
                (bA, tA), (bB, tB) = wis[2 * pair], wis[2 * pair + 1]
                mt = mtpool.tile([128, BLK * WPAD], mybir.dt.bfloat16)
                for half, (b, t) in enumerate(((bA, tA), (bB, tB))):
                    nc.sync.dma_start(
                        mt[64 * half:64 * half + 64, :],
                        bass.AP(xt,
                                _win_base(t) * PLANES * WPAD + BLK * b * WPAD,
                                [[PLANES * WPAD, WIN], [1, BLK * WPAD]]))

                if True:                    # one store-group of all 8 spans
                    evs = [evpool.tile([128, 8 * 512], mybir.dt.bfloat16,
                                       name=f"ev{h}") for h in range(2)]
                    for par in range(8):    # span index within the pair
                        k = par
                        pts = [pspool.tile([128, W], mybir.dt.float32,
                                           name=f"pt{h}") for h in range(2)]
                        for dw in range(7):
                            for half, t in enumerate((tA, tB)):
                                r = 64 * half
                                v = 1 if t == 8 else 0
                                for c in (0, 64):
                                    pl = k + 8 * (c // 64)
                                    nc.tensor.matmul(
                                        pts[half][c:c + 64, :],
                                        wb[r:r + 64,
                                           (v * 7 + dw) * 64:(v * 7 + dw + 1) * 64],
                                        mt[r:r + 64, pl * WPAD + dw:
                                           pl * WPAD + dw + W],
                                        start=(dw == 0), stop=(dw == 6),
                                        tile_position=(r, c))
                        # evict both banks: fp32 PSUM -> bf16 SBUF copies
                        for half in range(2):
                            dst = evs[half][:, par * 512:par * 512 + 512]
                            if (half + par) % 2 == 0:
                                nc.vector.tensor_copy(dst, pts[half][:, :])
                            else:
                                nc.scalar.copy(dst, pts[half][:, :])
                    # stores: 2 per bank (one per 64-partition half of the
                    # PSUM layout), 8KB descriptors; SWDGE + sync HWDGE split
                    for half, (b, t) in enumerate(((bA, tA), (bB, tB))):
                        mlo = TAIL_MLO if t == 8 else 0
                        nm = WOUT - mlo
                        row0 = _win_base(t) + mlo
                        for q in range(2):
                            eng = nc.gpsimd if half == 0 else nc.sync
                            eng.dma_start(
                                bass.AP(out,
                                        (row0 * PLANES + BLK * b
                                         + 8 * q) * W,
                                        [[PLANES * W, nm], [1, 8 * W]]),
                                evs[half][64 * q + mlo:64 * q + mlo + nm, :])

    nc.compile()
    _NC_CACHE["nc"] = nc
    return nc


def _prep_inputs(X, K, b, n_cores=N_CORES):
    Keff = np.asarray(K, np.float32).sum(axis=(0, 1))
    wt = _build_weights(Keff)
    Xr = np.asarray(X, np.float32).reshape(-1, H, W)
    X16 = Xr.astype(ml_dtypes.bfloat16)
    in_maps = []
    for i in range(n_cores):
        Xt = np.zeros((HPAD, PLANES, WPAD), ml_dtypes.bfloat16)
        Xt[3:3 + H, :, 3:3 + W] = X16[i * PLANES:(i + 1) * PLANES].transpose(1, 0, 2)
        in_maps.append({"xt": Xt, "wt": wt})
    bias = np.float32(np.asarray(b, np.float32).reshape(-1)[0]) * np.float32(
        np.asarray(K).size)
    return in_maps, bias


def kernel(X, K, b):
    in_maps, bias = _prep_inputs(X, K, b)
    nc = _get_module()
    res = run_bass_kernel_spmd(nc, in_maps, list(range(N_CORES)))
    shape = np.asarray(X).shape
    out = np.empty((N_CORES * PLANES, H, W), np.float32)
    for i in range(N_CORES):
        # res out: [H, PLANES, W] bf16 -> [PLANES, H, W] f32 + bias
        oc = np.asarray(res.results[i]["out"])
        out[i * PLANES:(i + 1) * PLANES] = (
            oc.transpose(1, 0, 2).astype(np.float32) + bias)
    return out.reshape(shape)


# revision 14
# speedup vs baseline: 1.6548x; 1.6548x over previous
"""Trainium2 Bass kernel for nn_Conv_39273180955616.

Computes, for X:(16,64,512,512) f32, K:(1,1,7,7), b:(1,1,1,1):
    out[n,c] = correlate2d(X[n,c], Keff, pad=3) + 49*b
where Keff = K.sum(axis=(0,1)).

Data parallel over the 1024 (n,c) planes -> 128 planes/core on 8 cores.

Per core the image planes are stored h-major in HBM (Xt[h, plane, w],
zero-padded to 518 in h and w) so one DMA descriptor carries a 16-plane
row run (16.6 KB).  The 7x7 correlation runs on TensorE as banded-
Toeplitz matmuls packed 4-per-pass onto the 128x128 PE array via 64x64
tile_position tiles: row half r in {0,64} holds a 64-row h-window (58
output rows), col half c in {0,64} computes a different plane, and the
7 kernel-column shifts accumulate in PSUM as free-dim offsets into the
518-wide padded rows.  Windows chain in pairs across the per-core
(block, window) list; the last window of each plane overlaps rows with
the previous one (band cols m<10 zeroed) so every window is full-size.
PSUM is evicted to SBUF as bf16 by DVE/ACT copies; stores go out over
the gpsimd SWDGE ring with 2 KB descriptors ([h, plane, w] bf16 layout,
plane pairs packed per partition).  The +49*b bias and the f32 cast are
applied on the host during the gather.
"""
import numpy as np
import ml_dtypes

import concourse.bass as bass
import concourse.tile as tile
from concourse import bacc, mybir
from concourse.bass_utils import run_bass_kernel_spmd

N_CORES = 8
H = 512
W = 512
HPAD = H + 6
WPAD = W + 6
PLANES = 16 * 64 // N_CORES     # 128 planes per core
BLK = 16                        # planes per block (descriptor run length)
NBLK = PLANES // BLK            # 8
WIN = 64                        # input rows per window
WOUT = 58                       # output rows per window (WIN - 6)
NWIN = 9                        # windows per plane: 8 @ stride 58 + 1 tail
TAIL_BASE = HPAD - WIN          # padded row base of tail window = 454
TAIL_MLO = 10                   # tail window valid outputs: m in [10, 58)


def _win_base(t):
    return 58 * t if t < 8 else TAIL_BASE


def _build_weights(Keff: np.ndarray) -> np.ndarray:
    """wb [128, 2*7*64] bf16: variant v (0 normal, 1 tail), dw in 0..6.

    B[p, m] = Keff[p-m, dw] for 0 <= p-m <= 6, m in [mlo, 58), else 0.
    Both 64-partition halves hold the same content.
    """
    wb = np.zeros((128, 2 * 7 * 64), np.float32)
    p = np.arange(64)[:, None]
    m = np.arange(64)[None, :]
    dh = p - m
    for v in range(2):
        mlo = TAIL_MLO if v == 1 else 0
        ok = (dh >= 0) & (dh < 7) & (m >= mlo) & (m < WOUT)
        for dw in range(7):
            blk = np.zeros((64, 64), np.float32)
            blk[ok] = Keff[dh[ok], dw]
            c0 = (v * 7 + dw) * 64
            wb[0:64, c0:c0 + 64] = blk
            wb[64:128, c0:c0 + 64] = blk
    return wb.astype(ml_dtypes.bfloat16)


_NC_CACHE = {}


def _get_module():
    if "nc" in _NC_CACHE:
        return _NC_CACHE["nc"]
    nc = bacc.Bacc("TRN2", target_bir_lowering=False, debug=False,
                   num_devices=N_CORES)
    xt = nc.dram_tensor("xt", [HPAD, PLANES, WPAD], mybir.dt.bfloat16,
                        kind="ExternalInput")
    wt = nc.dram_tensor("wt", [128, 2 * 7 * 64], mybir.dt.bfloat16,
                        kind="ExternalInput")
    out = nc.dram_tensor("out", [H, PLANES, W], mybir.dt.bfloat16,
                         kind="ExternalOutput")

    # flat (block, window) list; consecutive entries pair onto the two
    # 64-partition halves of one macrotile
    wis = [(b, t) for b in range(NBLK) for t in range(NWIN)]
    assert len(wis) % 2 == 0

    with tile.TileContext(nc) as tc:
        with (
            tc.tile_pool(name="wp", bufs=1) as wpool,
            tc.tile_pool(name="mt", bufs=3) as mtpool,
            tc.tile_pool(name="ps", bufs=3, space="PSUM") as pspool,
            tc.tile_pool(name="ev", bufs=3) as evpool,
        ):
            wb = wpool.tile([128, 2 * 7 * 64], mybir.dt.bfloat16)
            nc.sync.dma_start(wb[:], wt.ap())

            for pair in range(len(wis) // 2):
                (bA, tA), (bB, tB) = wis[2 * pair], wis[2 * pair + 1]
                mt = mtpool.tile([128, BLK * WPAD], mybir.dt.bfloat16)
                for half, (b, t) in enumerate(((bA, tA), (bB, tB))):
                    nc.sync.dma_start(
                        mt[64 * half:64 * half + 64, :],
                        bass.AP(xt,
                                _win_base(t) * PLANES * WPAD + BLK * b * WPAD,
                                [[PLANES * WPAD, WIN], [1, BLK * WPAD]]))

                if True:                    # one store-group of all 8 spans
                    evs = [evpool.tile([128, 8 * 512], mybir.dt.bfloat16,
                                       name=f"ev{h}") for h in range(2)]
                    for par in range(8):    # span index within the pair
                        k = par
                        pts = [pspool.tile([128, W], mybir.dt.float32,
                                           name=f"pt{h}") for h in range(2)]
                        for dw in range(7):
                            for half, t in enumerate((tA, tB)):
                                r = 64 * half
                                v = 1 if t == 8 else 0
                                for c in (0, 64):
                                    pl = k + 8 * (c // 64)
                                    nc.tensor.matmul(
                                        pts[half][c:c + 64, :],
                                        wb[r:r + 64,
                                           (v * 7 + dw) * 64:(v * 7 + dw + 1) * 64],
                                        mt[r:r + 64, pl * WPAD + dw:
                                           pl * WPAD + dw + W],
                                        start=(dw == 0), stop=(dw == 6),
                                        tile_position=(r, c))
                        # evict both banks: fp32 PSUM -> bf16 SBUF copies
                        for half in range(2):
                            dst = evs[half][:, par * 512:par * 512 + 512]
                            if (half + par) % 2 == 0:
                                nc.vector.tensor_copy(dst, pts[half][:, :])
                            else:
                                nc.scalar.copy(dst, pts[half][:, :])
                    # stores: 2 per bank (one per 64-partition half of the
                    # PSUM layout), 8KB descriptors; SWDGE + sync HWDGE split
                    for half, (b, t) in enumerate(((bA, tA), (bB, tB))):
                        mlo = TAIL_MLO if t == 8 else 0
                        nm = WOUT - mlo
                        row0 = _win_base(t) + mlo
                        for q in range(2):
                            eng = nc.gpsimd
                            eng.dma_start(
                                bass.AP(out,
                                        (row0 * PLANES + BLK * b
                                         + 8 * q) * W,
                                        [[PLANES * W, nm], [1, 8 * W]]),
                                evs[half][64 * q + mlo:64 * q + mlo + nm, :])

    nc.compile()
    _NC_CACHE["nc"] = nc
    return nc


def _prep_inputs(X, K, b, n_cores=N_CORES):
    Keff = np.asarray(K, np.float32).sum(axis=(0, 1))
    wt = _build_weights(Keff)
    Xr = np.asarray(X, np.float32).reshape(-1, H, W)
    X16 = Xr.astype(ml_dtypes.bfloat16)
    in_maps = []
    for i in range(n_cores):
        Xt = np.zeros((HPAD, PLANES, WPAD), ml_dtypes.bfloat16)
        Xt[3:3 + H, :, 3:3 + W] = X16[i * PLANES:(i + 1) * PLANES].transpose(1, 0, 2)
        in_maps.append({"xt": Xt, "wt": wt})
    bias = np.float32(np.asarray(b, np.float32).reshape(-1)[0]) * np.float32(
        np.asarray(K).size)
    return in_maps, bias


def kernel(X, K, b):
    in_maps, bias = _prep_inputs(X, K, b)
    nc = _get_module()
    res = run_bass_kernel_spmd(nc, in_maps, list(range(N_CORES)))
    shape = np.asarray(X).shape
    out = np.empty((N_CORES * PLANES, H, W), np.float32)
    for i in range(N_CORES):
        # res out: [H, PLANES, W] bf16 -> [PLANES, H, W] f32 + bias
        oc = np.asarray(res.results[i]["out"])
        out[i * PLANES:(i + 1) * PLANES] = (
            oc.transpose(1, 0, 2).astype(np.float32) + bias)
    return out.reshape(shape)


# revision 15
# speedup vs baseline: 1.7167x; 1.0374x over previous
"""Trainium2 Bass kernel for nn_Conv_39273180955616.

Computes, for X:(16,64,512,512) f32, K:(1,1,7,7), b:(1,1,1,1):
    out[n,c] = correlate2d(X[n,c], Keff, pad=3) + 49*b
where Keff = K.sum(axis=(0,1)).

Data parallel over the 1024 (n,c) planes -> 128 planes/core on 8 cores.

Per core the image planes are stored h-major in HBM (Xt[h, plane, w],
zero-padded to 518 in h and w) so one DMA descriptor carries a 16-plane
row run (16.6 KB).  The 7x7 correlation runs on TensorE as banded-
Toeplitz matmuls packed 4-per-pass onto the 128x128 PE array via 64x64
tile_position tiles: row half r in {0,64} holds a 64-row h-window (58
output rows), col half c in {0,64} computes a different plane, and the
7 kernel-column shifts accumulate in PSUM as free-dim offsets into the
518-wide padded rows.  Windows chain in pairs across the per-core
(block, window) list; the last window of each plane overlaps rows with
the previous one (band cols m<10 zeroed) so every window is full-size.
PSUM is evicted to SBUF as bf16 by DVE/ACT copies; stores go out over
the gpsimd SWDGE ring with 2 KB descriptors ([h, plane, w] bf16 layout,
plane pairs packed per partition).  The +49*b bias and the f32 cast are
applied on the host during the gather.
"""
import numpy as np
import ml_dtypes

import concourse.bass as bass
import concourse.tile as tile
from concourse import bacc, mybir
from concourse.bass_utils import run_bass_kernel_spmd

N_CORES = 8
H = 512
W = 512
HPAD = H + 6
WPAD = W + 6
PLANES = 16 * 64 // N_CORES     # 128 planes per core
BLK = 16                        # planes per block (descriptor run length)
NBLK = PLANES // BLK            # 8
WIN = 64                        # input rows per window
WOUT = 58                       # output rows per window (WIN - 6)
NWIN = 9                        # windows per plane: 8 @ stride 58 + 1 tail
TAIL_BASE = HPAD - WIN          # padded row base of tail window = 454
TAIL_MLO = 10                   # tail window valid outputs: m in [10, 58)


def _win_base(t):
    return 58 * t if t < 8 else TAIL_BASE


def _build_weights(Keff: np.ndarray) -> np.ndarray:
    """wb [128, 2*7*64] bf16: variant v (0 normal, 1 tail), dw in 0..6.

    B[p, m] = Keff[p-m, dw] for 0 <= p-m <= 6, m in [mlo, 58), else 0.
    Both 64-partition halves hold the same content.
    """
    wb = np.zeros((128, 2 * 7 * 64), np.float32)
    p = np.arange(64)[:, None]
    m = np.arange(64)[None, :]
    dh = p - m
    for v in range(2):
        mlo = TAIL_MLO if v == 1 else 0
        ok = (dh >= 0) & (dh < 7) & (m >= mlo) & (m < WOUT)
        for dw in range(7):
            blk = np.zeros((64, 64), np.float32)
            blk[ok] = Keff[dh[ok], dw]
            c0 = (v * 7 + dw) * 64
            wb[0:64, c0:c0 + 64] = blk
            wb[64:128, c0:c0 + 64] = blk
    return wb.astype(ml_dtypes.bfloat16)


_NC_CACHE = {}


def _get_module():
    if "nc" in _NC_CACHE:
        return _NC_CACHE["nc"]
    nc = bacc.Bacc("TRN2", target_bir_lowering=False, debug=False,
                   num_devices=N_CORES)
    xt = nc.dram_tensor("xt", [HPAD, PLANES, WPAD], mybir.dt.bfloat16,
                        kind="ExternalInput")
    wt = nc.dram_tensor("wt", [128, 2 * 7 * 64], mybir.dt.bfloat16,
                        kind="ExternalInput")
    out = nc.dram_tensor("out", [H, PLANES, W], mybir.dt.bfloat16,
                         kind="ExternalOutput")

    # flat (block, window) list; consecutive entries pair onto the two
    # 64-partition halves of one macrotile
    wis = [(b, t) for b in range(NBLK) for t in range(NWIN)]
    assert len(wis) % 2 == 0

    with tile.TileContext(nc) as tc:
        with (
            tc.tile_pool(name="wp", bufs=1) as wpool,
            tc.tile_pool(name="mt", bufs=3) as mtpool,
            tc.tile_pool(name="ps", bufs=4, space="PSUM") as pspool,
            tc.tile_pool(name="ev", bufs=5) as evpool,
        ):
            wb = wpool.tile([128, 2 * 7 * 64], mybir.dt.bfloat16)
            nc.sync.dma_start(wb[:], wt.ap())

            for pair in range(len(wis) // 2):
                (bA, tA), (bB, tB) = wis[2 * pair], wis[2 * pair + 1]
                mt = mtpool.tile([128, BLK * WPAD], mybir.dt.bfloat16)
                for half, (b, t) in enumerate(((bA, tA), (bB, tB))):
                    nc.sync.dma_start(
                        mt[64 * half:64 * half + 64, :],
                        bass.AP(xt,
                                _win_base(t) * PLANES * WPAD + BLK * b * WPAD,
                                [[PLANES * WPAD, WIN], [1, BLK * WPAD]]))

                if True:                    # one store-group of all 8 spans
                    evs = [evpool.tile([128, 8 * 512], mybir.dt.bfloat16,
                                       name=f"ev{h}") for h in range(2)]
                    for par in range(8):    # span index within the pair
                        k = par
                        pts = [pspool.tile([128, W], mybir.dt.float32,
                                           name=f"pt{h}") for h in range(2)]
                        for dw in range(7):
                            for half, t in enumerate((tA, tB)):
                                r = 64 * half
                                v = 1 if t == 8 else 0
                                for c in (0, 64):
                                    pl = k + 8 * (c // 64)
                                    nc.tensor.matmul(
                                        pts[half][c:c + 64, :],
                                        wb[r:r + 64,
                                           (v * 7 + dw) * 64:(v * 7 + dw + 1) * 64],
                                        mt[r:r + 64, pl * WPAD + dw:
                                           pl * WPAD + dw + W],
                                        start=(dw == 0), stop=(dw == 6),
                                        tile_position=(r, c))
                        # evict both banks: fp32 PSUM -> bf16 SBUF copies
                        for half in range(2):
                            dst = evs[half][:, par * 512:par * 512 + 512]
                            if (half + par) % 2 == 0:
                                nc.vector.tensor_copy(dst, pts[half][:, :])
                            else:
                                nc.scalar.copy(dst, pts[half][:, :])
                    # stores: 2 per bank (one per 64-partition half of the
                    # PSUM layout), 8KB descriptors; SWDGE + sync HWDGE split
                    for half, (b, t) in enumerate(((bA, tA), (bB, tB))):
                        mlo = TAIL_MLO if t == 8 else 0
                        nm = WOUT - mlo
                        row0 = _win_base(t) + mlo
                        for q in range(2):
                            eng = nc.gpsimd
                            eng.dma_start(
                                bass.AP(out,
                                        (row0 * PLANES + BLK * b
                                         + 8 * q) * W,
                                        [[PLANES * W, nm], [1, 8 * W]]),
                                evs[half][64 * q + mlo:64 * q + mlo + nm, :])

    nc.compile()
    _NC_CACHE["nc"] = nc
    return nc


def _prep_inputs(X, K, b, n_cores=N_CORES):
    Keff = np.asarray(K, np.float32).sum(axis=(0, 1))
    wt = _build_weights(Keff)
    Xr = np.asarray(X, np.float32).reshape(-1, H, W)
    X16 = Xr.astype(ml_dtypes.bfloat16)
    in_maps = []
    for i in range(n_cores):
        Xt = np.zeros((HPAD, PLANES, WPAD), ml_dtypes.bfloat16)
        Xt[3:3 + H, :, 3:3 + W] = X16[i * PLANES:(i + 1) * PLANES].transpose(1, 0, 2)
        in_maps.append({"xt": Xt, "wt": wt})
    bias = np.float32(np.asarray(b, np.float32).reshape(-1)[0]) * np.float32(
        np.asarray(K).size)
    return in_maps, bias


def kernel(X, K, b):
    in_maps, bias = _prep_inputs(X, K, b)
    nc = _get_module()
    res = run_bass_kernel_spmd(nc, in_maps, list(range(N_CORES)))
    shape = np.asarray(X).shape
    out = np.empty((N_CORES * PLANES, H, W), np.float32)
    for i in range(N_CORES):
        # res out: [H, PLANES, W] bf16 -> [PLANES, H, W] f32 + bias
        oc = np.asarray(res.results[i]["out"])
        out[i * PLANES:(i + 1) * PLANES] = (
            oc.transpose(1, 0, 2).astype(np.float32) + bias)
    return out.reshape(shape)
